# revision 18
# baseline (speedup 1.0000x reference)
"""BMMRemapper Trainium2 kernel (v6: 4-queue dma_gather, int8 table, bf16
mega-op combine).

Math: out[n,c,q] = sum_k x[n,c,k] * mat[n,q,k] where mat is the bilinear
interpolation matrix built from grid (4 nonzeros per row q: rows lin, lin+1,
lin+48, lin+49 of x^T with weights (1-a)(1-b), (1-a)b, a(1-b), ab).

The host stages a quad-row table xq[k] = [xT[k], xT[k+1], xT[k+48], xT[k+49]]
quantized to int8 with one global scale per batch (pure data movement +
dtype cast), so one 512-byte gather descriptor per output pixel fetches all
four corner rows. The 18 gather tiles are issued as 8 dma_gather
instructions spread over the 4 SWDGE queues; each queue is serviced by its
own pair of GPSIMD Q7 cores, so descriptor generation runs 4-way parallel
(~11.5ns/descriptor per pair).

floor(x) for x in [EPS, 47) = i32_rne_cast(x - 0.5): one tensor_scalar with
i32 output dtype. Exact-integer ties resolve to (base-1, frac 1.0), which
yields the identical interpolated value (bilinear continuity).

Combine: the otherwise-idle ACT engine computes corner j=0's product per
tile (activation Copy with per-partition f32 scale); the DVE does corners
j=1..3 with one big strided TT per span: prod = bcast(cof4_bf16) * dest_i8
(stride-0 AP dim on channels, mixed-dtype mult), then the add tree
x1=p1+p2, x2=u0+p3, out=x1+x2, with add spans interleaved into the prod
stream so they fill DVE stalls while the second gather round is in flight.
Gathers run as 2 rounds per queue (each extra instruction on a queue pays
~2.2us of non-pipelined completion latency). The int8 dequant scale and
the disk mask are both folded into the coefficients.

Sharding: batch-parallel, one batch per NeuronCore (N=8), no cross-core
communication. The disk mask couples batches (all-batch AND), so every core
receives the full grid (gall) and computes the mask locally.

Layouts (q = output pixel, 0..2303; t = q//128; p = q%128):
  xq     (2304, 512) int8 : quad-row table (row k -> 4 corner rows, lin=k).
  gwrap  (128, 288)  f32  : wrapped coords for dma_gather's int16 index
                            layout, [p, 2s+c] = grid[16s+(p%16), c].
  gcoef  (128, 36)   f32  : own-batch grid, [p, 2t+coord].
  gall   (128, 288)  f32  : all-batch grid, [p, 16t+2m+coord].
  dqs    (128, 1)    f32  : int8 dequant scale (max|x_n|/127, replicated).
  outp   (128, 2304) bf16 : [p, t*128 + c]  (host re-permutes + upcasts).
"""

import numpy as np
import ml_dtypes

N, H, W, C = 8, 48, 48, 128
HW = H * W            # 2304
NT = HW // 128        # 18
NWRAP = HW // 16      # 144
EPS = 1e-5
CLIP_HI = float(np.float32(float(H - 1) - EPS))  # 46.99999 (f32)

# (t0, t1, queue): dma_gather groups, 4-way queue parallel; small leading
# and trailing groups so combine products start early and the tail is short
GG = [
    (0, 2, 0), (2, 4, 1), (4, 6, 2), (6, 8, 3),
    (8, 10, 0), (10, 13, 1), (13, 16, 2), (16, 18, 3),
]
ADD_SPANS = [(0, 8), (8, 13), (13, 18)]

_CACHE = {}


def _build_nc():
    from contextlib import ExitStack

    import concourse.bacc as bacc
    import concourse.mybir as mybir
    import concourse.tile as tile
    from concourse.library_config import mlp

    dt = mybir.dt
    f32, i32, i16 = dt.float32, dt.int32, dt.int16
    bf16 = dt.bfloat16
    i8 = dt.int8
    Alu = mybir.AluOpType
    Act = mybir.ActivationFunctionType

    nc = bacc.Bacc(
        "TRN2",
        target_bir_lowering=False,
        debug=False,
        num_devices=N,
        num_swdge_queues=4,
    )

    xq = nc.dram_tensor("xq", [HW, 4 * C], i8, kind="ExternalInput")
    gwrap = nc.dram_tensor("gwrap", [128, 2 * NWRAP], f32, kind="ExternalInput")
    gcoef = nc.dram_tensor("gcoef", [128, 2 * NT], f32, kind="ExternalInput")
    gall = nc.dram_tensor("gall", [128, 16 * NT], f32, kind="ExternalInput")
    dqs = nc.dram_tensor("dqs", [128, 1], f32, kind="ExternalInput")
    outp = nc.dram_tensor("outp", [128, HW], bf16, kind="ExternalOutput")

    with tile.TileContext(nc) as tc, ExitStack() as ctx:
        pool = ctx.enter_context(tc.tile_pool(name="p", bufs=1))

        nc.gpsimd.load_library(mlp)

        # ---- loads (HWDGE): gwrap gates the gathers -> sync queue first ----
        g_wrap = pool.tile([128, 2 * NWRAP], f32)
        nc.sync.dma_start(g_wrap[:], gwrap.ap())
        g_coef = pool.tile([128, 2 * NT], f32)
        nc.sync.dma_start(g_coef[:], gcoef.ap())
        g_all = pool.tile([128, 16 * NT], f32)
        nc.scalar.dma_start(g_all[:], gall.ap())
        dq = pool.tile([128, 1], f32)
        nc.scalar.dma_start(dq[:], dqs.ap())

        # ---- wrapped idx chain [128, 144] (feeds dma_gather) ----
        wa = pool.tile([128, NWRAP], f32)
        wb = pool.tile([128, NWRAP], f32)
        nc.vector.tensor_scalar(wa[:], g_wrap[:, 0::2], EPS, CLIP_HI, Alu.max, Alu.min)
        nc.vector.tensor_scalar(wb[:], g_wrap[:, 1::2], EPS, CLIP_HI, Alu.max, Alu.min)
        iwa = pool.tile([128, NWRAP], i32)
        iwb = pool.tile([128, NWRAP], i32)
        nc.vector.tensor_scalar(iwa[:], wa[:], -0.5, None, Alu.add)
        nc.vector.tensor_scalar(iwb[:], wb[:], -0.5, None, Alu.add)
        lin16 = pool.tile([128, NWRAP], i16)
        nc.vector.scalar_tensor_tensor(lin16[:], iwa[:], W, iwb[:], Alu.mult, Alu.add)

        # ---- gathers: 8 dma_gather instructions on 4 queues ----
        dest = pool.tile([128, NT * 4 * C], i8)
        d3 = dest[:].rearrange("p (t e) -> p t e", e=4 * C)
        for (t0, t1, qn) in GG:
            nidx = 128 * (t1 - t0)
            nc.gpsimd.dma_gather(
                d3[:, t0:t1, :],
                xq.ap(),
                lin16[:, 8 * t0 : 8 * t1],
                nidx,
                nidx,
                4 * C,
                queue_num=qn,
            )

        # ---- coef chain [128, 18]; ACT handles the affine pieces ----
        ca = pool.tile([128, NT], f32)
        cb = pool.tile([128, NT], f32)
        nc.vector.tensor_scalar(ca[:], g_coef[:, 0::2], EPS, CLIP_HI, Alu.max, Alu.min)
        nc.vector.tensor_scalar(cb[:], g_coef[:, 1::2], EPS, CLIP_HI, Alu.max, Alu.min)
        ia = pool.tile([128, NT], i32)
        ib = pool.tile([128, NT], i32)
        nc.vector.tensor_scalar(ia[:], ca[:], -0.5, None, Alu.add)
        nc.vector.tensor_scalar(ib[:], cb[:], -0.5, None, Alu.add)
        af = pool.tile([128, NT], f32)
        bf = pool.tile([128, NT], f32)
        nc.scalar.activation(af[:], ia[:], Act.Copy)
        nc.scalar.activation(bf[:], ib[:], Act.Copy)
        fa = pool.tile([128, NT], f32)   # frac a
        fb = pool.tile([128, NT], f32)   # frac b
        nc.vector.tensor_tensor(fa[:], ca[:], af[:], Alu.subtract)
        nc.vector.tensor_tensor(fb[:], cb[:], bf[:], Alu.subtract)

        # mask: AND over all batches of in-bounds test
        g_all3 = g_all[:].rearrange("p (t m) -> p t m", m=16)
        mn = pool.tile([128, NT], f32)
        mx = pool.tile([128, NT], f32)
        nc.vector.tensor_reduce(mn[:], g_all3, mybir.AxisListType.X, Alu.min)
        nc.vector.tensor_reduce(mx[:], g_all3, mybir.AxisListType.X, Alu.max)
        mge = pool.tile([128, NT], f32)
        mle = pool.tile([128, NT], f32)
        nc.vector.tensor_scalar(mge[:], mn[:], -0.5, None, Alu.is_ge)
        nc.vector.tensor_scalar(mle[:], mx[:], float(H) - 0.5, None, Alu.is_le)
        mask = pool.tile([128, NT], f32)
        nc.vector.tensor_tensor(mask[:], mge[:], mle[:], Alu.mult)

        fb0 = pool.tile([128, NT], f32)   # 1-b
        fa0 = pool.tile([128, NT], f32)   # 1-a
        nc.scalar.activation(fb0[:], fb[:], Act.Copy, scale=-1.0, bias=1.0)
        nc.scalar.activation(fa0[:], fa[:], Act.Copy, scale=-1.0, bias=1.0)
        # (1-a)*mask*dq and a*mask*dq: fold int8 dequant scale in for free
        fa0m = pool.tile([128, NT], f32)
        fa1m = pool.tile([128, NT], f32)
        nc.vector.scalar_tensor_tensor(fa0m[:], fa0[:], dq[:], mask[:], Alu.mult, Alu.mult)
        nc.vector.scalar_tensor_tensor(fa1m[:], fa[:], dq[:], mask[:], Alu.mult, Alu.mult)

        # interleaved coefficient tile: cof4[p, 4t+j] = c_j[p, t] (bf16)
        cof4 = pool.tile([128, 4 * NT], bf16)
        c4 = cof4[:].rearrange("p (t j) -> p t j", j=4)
        c00f = pool.tile([128, NT], f32)      # j=0 coef for the ACT products
        nc.vector.tensor_tensor(c00f[:], fa0m[:], fb0[:], Alu.mult)
        nc.vector.tensor_tensor(c4[:, :, 1], fa0m[:], fb[:], Alu.mult)
        nc.vector.tensor_tensor(c4[:, :, 2], fa1m[:], fb0[:], Alu.mult)
        nc.vector.tensor_tensor(c4[:, :, 3], fa1m[:], fb[:], Alu.mult)

        # ---- combine (mega-ops): prod = dest_i8 * bcast(cof4); adds ----
        prod = pool.tile([128, NT * 4 * C], bf16)
        p4 = prod[:].rearrange("p (t j c) -> p t j c", j=4, c=C)
        d4 = dest[:].rearrange("p (t j c) -> p t j c", j=4, c=C)
        out_sb = pool.tile([128, HW], bf16)
        s01 = pool.tile([128, NT * 2 * C], bf16)
        s3 = s01[:].rearrange("p (t j c) -> p t j c", j=2, c=C)
        o3 = out_sb[:].rearrange("p (t c) -> p t c", c=C)
        # ACT: u0[t] = c00 * dest[t, j=0]  (otherwise-idle scalar engine)
        p0 = pool.tile([128, NT * C], bf16)
        for t in range(NT):
            nc.scalar.activation(
                p0[:, t * C : (t + 1) * C],
                dest[:, (4 * t) * C : (4 * t + 1) * C],
                Act.Copy,
                scale=c00f[:, t : t + 1],
            )
        p0v = p0[:].rearrange("p (t c) -> p t c", c=C)

        def emit_prod(t0, t1):
            cbc = (
                c4[:, t0:t1, 1:4]
                .unsqueeze(3)
                .broadcast_to([128, t1 - t0, 3, C])
            )
            nc.vector.tensor_tensor(
                p4[:, t0:t1, 1:4, :], cbc, d4[:, t0:t1, 1:4, :], Alu.mult
            )

        def emit_adds(t0, t1):
            nc.vector.tensor_tensor(
                s3[:, t0:t1, 0, :], p4[:, t0:t1, 1, :], p4[:, t0:t1, 2, :], Alu.add
            )
            nc.vector.tensor_tensor(
                s3[:, t0:t1, 1, :], p0v[:, t0:t1], p4[:, t0:t1, 3, :], Alu.add
            )
            nc.vector.tensor_tensor(
                o3[:, t0:t1], s3[:, t0:t1, 0, :], s3[:, t0:t1, 1, :], Alu.add
            )

        emit_prod(0, 2); emit_prod(2, 4); emit_prod(4, 6); emit_prod(6, 8)
        emit_adds(0, 8)
        emit_prod(8, 10); emit_prod(10, 13)
        emit_adds(8, 13)
        emit_prod(13, 16); emit_prod(16, 18)
        emit_adds(13, 18)

        # ---- stores ----
        for (c0, c1) in ADD_SPANS:
            nc.sync.dma_start(
                outp.ap()[:, c0 * C : c1 * C],
                out_sb[:, c0 * C : c1 * C],
            )

    nc.compile()
    return nc


def _get_nc():
    if "nc" not in _CACHE:
        _CACHE["nc"] = _build_nc()
    return _CACHE["nc"]


def _stage_inputs(x, grid):
    """Per-core input maps (data movement / replication / dtype cast)."""
    x = np.ascontiguousarray(x, dtype=np.float32)
    grid = np.ascontiguousarray(grid, dtype=np.float32)
    xr = x.reshape(N, C, HW)
    gr = grid.reshape(N, HW, 2)

    # quad-row table: xq[n][k] = [xT[k], xT[k+1], xT[k+48], xT[k+49]],
    # quantized int8 with one global scale per batch
    xt = np.zeros((N, HW + W + 2, C), dtype=np.float32)
    xt[:, :HW] = xr.transpose(0, 2, 1)
    xqf = np.empty((N, HW, 4 * C), dtype=np.float32)
    xqf[:, :, 0 * C : 1 * C] = xt[:, 0:HW]
    xqf[:, :, 1 * C : 2 * C] = xt[:, 1 : HW + 1]
    xqf[:, :, 2 * C : 3 * C] = xt[:, W : HW + W]
    xqf[:, :, 3 * C : 4 * C] = xt[:, W + 1 : HW + W + 1]
    s = np.abs(xr).max(axis=(1, 2)).astype(np.float32)   # [N]
    s = np.maximum(s, 1e-30)
    xq8 = np.clip(
        np.round(xqf / s[:, None, None] * 127.0), -127, 127
    ).astype(np.int8)
    dqv = (s / np.float32(127.0)).astype(np.float32)      # [N]
    dqs = np.tile(dqv[:, None, None], (1, 128, 1))        # [N, 128, 1]

    # gwrap[n][p, 2s+c] = gr[n, 16s + (p%16), c]  (replicated mod 16)
    gw = gr.reshape(N, NWRAP, 16, 2)                      # [n, s, p16, c]
    gw = gw.transpose(0, 2, 1, 3).reshape(N, 16, 2 * NWRAP)
    gwrap = np.ascontiguousarray(np.tile(gw, (1, 8, 1))).reshape(N, 128, 2 * NWRAP)

    # gcoef[n][p, 2t+c] = gr[n, t*128+p, c]
    gc = gr.reshape(N, NT, 128, 2).transpose(0, 2, 1, 3)  # [n, p, t, c]
    gcoef = np.ascontiguousarray(gc.reshape(N, 128, 2 * NT))

    # gall[p, 16t+2m+c] = gr[m, t*128+p, c]   (same for all cores)
    ga = gr.reshape(N, NT, 128, 2).transpose(2, 1, 0, 3)  # [p, t, m, c]
    gall = np.ascontiguousarray(ga.reshape(128, 16 * NT))

    return [
        {"xq": xq8[n], "gwrap": gwrap[n], "gcoef": gcoef[n], "gall": gall,
         "dqs": dqs[n]}
        for n in range(N)
    ]


def _unstage_output(results):
    """results[n]["outp"] is (128, 2304) bf16 = [p, t*128+c] -> (N, C, H, W)."""
    out = np.empty((N, C, H, W), dtype=np.float32)
    for n in range(N):
        o = np.asarray(results[n]["outp"]).astype(np.float32)
        o = o.reshape(128, NT, C)                        # [p, t, c]
        out[n] = o.transpose(2, 1, 0).reshape(C, H, W)   # [c, q=t*128+p]
    return out


def kernel(x, grid):
    from concourse import bass_utils

    nc = _get_nc()
    in_maps = _stage_inputs(x, grid)
    res = bass_utils.run_bass_kernel_spmd(nc, in_maps, core_ids=list(range(N)))
    return _unstage_output(res.results)
